# revision 3
# baseline (speedup 1.0000x reference)
"""FFM layer kernel for Trainium2, data-parallel over batch on 8 NeuronCores.

The reference computes, for each sample b:
    x = [dense(13) | onehot(26 fields x 1000)]            # [B, 26013]
    linear = w0 + x @ w                                   # [B, 1]
    field_f = einsum('bf,fik->bik', x, v)                 # [B, 39, 8]
    inter = 0.5*((sum_i field_f)^2.sum(k) - (field_f^2).sum(i,k))
    out = linear + inter

Because x is one-hot in the sparse block, x @ [v|w] is a 26-row gather from
an int8 [26013, 512] table (cols 0..311 = flattened v row / step, col 312 =
w / step, rest pad to a 256 B multiple) plus a tiny fp32 dense [14]x[14,314]
matmul in 1/step units (vdx pre-divided by step on the host; row 13 = ones
row carrying w0/step into col 312).  Each core handles 512 samples as 4
tiles of 128; each tile-half's 13 rows/sample come from ONE dma_gather call
(1664 descriptors) on a rotating SWDGE queue.

Startup is dominated by the gpsimd mlp-library reload (~10.6 us, fixed),
which gates the first gather; the mega input DMA and the dense matmuls
hide under it.  The int8 table halves wire bytes vs fp16 (512 B vs 768 B
rows); quantization uses a 4-sigma clipped step for accuracy margin
(rel_fro ~1.1e-2 vs the 2e-2 gate).

The 26 gathered rows are summed with a pairwise tree on DVE (level 1 reads
int8 at 1 elem/cycle - unavoidable, TRN2 DVE has no 8-bit packing - and the
upper levels run fp16 2x).  Everything stays in integer "1/step" units until
the very end: tot' = u + dnt', h1' = act-square-accum(tot'), r = (h2'-h1')
* 0.5*step^2, out = tot'[312]*step + r.  The [128, 4] per-tile outputs are
stored directly and un-permuted on the host (no PE transpose pass).
"""

import os

import numpy as np

N_DENSE = 13
N_SPARSE = 26
ONEHOT = 1000
FIELD = 39
K = 8
FEAT = N_DENSE + N_SPARSE * ONEHOT  # 26013
B = 4096
NCORES = 8
BC = B // NCORES  # 512 samples per core
P = 128
NT = BC // P  # 4 tiles per core
D = FIELD * K  # 312
DW = D + 1  # 313 (col 312 carries the linear weight)
DP = D + 2  # 314 (even payload width; col 313 zero pad)
NI = N_SPARSE * P  # 3328 gathered rows per tile
HC = (NI // 2) // 16  # idx columns per 13-field half (104)

TABLE_F16 = os.environ.get("K_F16", "0") == "1"
CLIP_SIGMA = float(os.environ.get("K_CLIP", "4.0"))
K_SCR = int(os.environ.get("K_SCR", "65536"))  # SWDGE desc ring carveout
E = 384 if TABLE_F16 else 512  # gathered row width in elements (=bytes for i8)

# mega input layout (bytes per partition)
MB_IDX = 0                    # [128, 832] int16 = 1664 B
MB_DNT = 1664                 # [14, 512] fp32 = 2048 B
MB_VDX = MB_DNT + 2048        # [14, 314] fp32 = 1256 B
MB = MB_VDX + 1256            # 4968

_cached = {}


def _build_program(step: float):
    key = step
    if key in _cached:
        return _cached[key]

    import concourse.bacc as bacc
    import concourse.mybir as mybir

    nc = bacc.Bacc(
        "TRN2",
        debug=False,
        enable_asserts=False,
        target_bir_lowering=False,
        num_devices=NCORES,
        num_swdge_queues=4,
        dynamic_dma_scratch_size=K_SCR,
    )
    f32 = mybir.dt.float32
    f16 = mybir.dt.float16
    i16 = mybir.dt.int16
    u8 = mybir.dt.uint8
    tdt = f16 if TABLE_F16 else mybir.dt.int8
    add_op = mybir.AluOpType.add
    mult_op = mybir.AluOpType.mult

    table = nc.dram_tensor("table", [FEAT, E], tdt, kind="ExternalInput").ap()
    mega = nc.dram_tensor("mega", [P, MB], u8, kind="ExternalInput").ap()
    out = nc.dram_tensor("out", [P, NT], f32, kind="ExternalOutput").ap()

    mega_sb = nc.alloc_sbuf_tensor("mega_sb", [P, MB], u8).ap()
    idx_sb = mega_sb[:, MB_IDX:MB_IDX + 1664].bitcast(i16)        # [128, 832]
    dnt_sb = mega_sb[0:N_DENSE + 1, MB_DNT:MB_DNT + 2048].bitcast(f32)   # [14, 512]
    vdx_sb = mega_sb[0:N_DENSE + 1, MB_VDX:MB_VDX + 1256].bitcast(f32)   # [14, 314]

    g_sb = [
        [nc.alloc_sbuf_tensor(f"g{t}_{h}", [P, 13 * E], tdt).ap() for h in range(2)]
        for t in range(NT)
    ]
    # two alternating fp16 partial-sum sets (vector-engine-private; explicit
    # vv sems serialize same-engine RAW reuse)
    a_sb = [
        [nc.alloc_sbuf_tensor(f"a{s}_{h}", [P, 6 * DP], f16).ap() for h in range(2)]
        for s in range(2)
    ]
    u_sb = [nc.alloc_sbuf_tensor(f"u{s}", [P, DP], f16).ap() for s in range(2)]
    dnt16_sb = nc.alloc_sbuf_tensor("dnt16", [P, NT * DP], f16).ap()
    tot_sb = [nc.alloc_sbuf_tensor(f"tot{t}", [P, DP], f16).ap() for t in range(NT)]
    s8_sb = [nc.alloc_sbuf_tensor(f"s8_{t}", [P, K], f32).ap() for t in range(NT)]
    sq8_sb = [nc.alloc_sbuf_tensor(f"sq8_{s}", [P, K], f32).ap() for s in range(2)]
    sq_sb = nc.alloc_sbuf_tensor("sq", [P, D], f16).ap()
    h1_sb = [nc.alloc_sbuf_tensor(f"h1_{t}", [P, 1], f32).ap() for t in range(NT)]
    h2_sb = [nc.alloc_sbuf_tensor(f"h2_{t}", [P, 1], f32).ap() for t in range(NT)]
    rr_sb = [nc.alloc_sbuf_tensor(f"rr_{t}", [P, 1], f32).ap() for t in range(NT)]
    ot4_sb = nc.alloc_sbuf_tensor("ot4", [P, NT], f32).ap()
    ps_ps = [nc.alloc_psum_tensor(f"ps{t}", [P, DP], f32).ap() for t in range(NT)]

    io = nc.alloc_semaphore("io")      # mega load x 16
    st = nc.alloc_semaphore("st")      # output store x 16
    # one sem per (tile, half) gather call (a DMA sem may only be updated
    # from one SWDGE queue)
    gs = [[nc.alloc_semaphore(f"gs{t}_{h}") for h in range(2)] for t in range(NT)]
    mm = nc.alloc_semaphore("mm")      # dense matmul done (per tile)
    ac = nc.alloc_semaphore("ac")      # Act PSUM->fp16 copy done (per tile)
    ah = nc.alloc_semaphore("ah")      # Act h1 accum done (per tile)
    dn = nc.alloc_semaphore("dn")      # ot column ready (per tile)
    vv = nc.alloc_semaphore("vv")      # vector-engine same-engine RAW ordering

    VOPS = 14  # vv increments per tile (op 15 increments dn instead)

    with nc.Block() as block:

        @block.sync
        def _(sync):
            sync.dma_start(mega_sb[:], mega[:]).then_inc(io, 16)
            sync.wait_ge(dn, NT)
            sync.dma_start(out[:], ot4_sb[:]).then_inc(st, 16)
            sync.wait_ge(st, 16)

        @block.gpsimd
        def _(gpsimd):
            from concourse import library_config as lc

            gpsimd.load_library(lc.mlp)
            gpsimd.wait_ge(io, 16)
            for t in range(NT):
                for h in range(2):
                    g3 = g_sb[t][h].rearrange("p (c e) -> p c e", e=E)
                    col = (2 * t + h) * HC
                    gpsimd.dma_gather(
                        g3[:, :, :],
                        table[:],
                        idx_sb[:, col:col + HC],
                        13 * P,
                        13 * P,
                        E,
                        single_packet=False,
                        queue_num=(2 * t + h) % 4,
                    ).then_inc(gs[t][h], 16)

        @block.tensor
        def _(tensor):
            tensor.wait_ge(io, 16)
            for t in range(NT):
                nc.tensor.matmul(
                    out=ps_ps[t][:],
                    lhsT=dnt_sb[:, t * P:(t + 1) * P],
                    rhs=vdx_sb[:],
                    start=True,
                    stop=True,
                ).then_inc(mm, 1)

        @block.scalar
        def _(scalar):
            copyf = mybir.ActivationFunctionType.Copy
            sqf = mybir.ActivationFunctionType.Square
            # downcast the dense-in-1/step-units PSUM to fp16 so the vector
            # add stays in the all-SBUF 2-byte fast path
            for t in range(NT):
                scalar.wait_ge(mm, t + 1)
                nc.scalar.activation(
                    out=dnt16_sb[:, t * DP:(t + 1) * DP], in_=ps_ps[t][:],
                    func=copyf,
                ).then_inc(ac, 1)
            # h1' = sum_i,k f'_ik^2 via the Act accumulator, off the DVE tail
            for t in range(NT):
                scalar.wait_ge(vv, VOPS * t + 10)
                nc.scalar.activation(
                    out=sq_sb[:], in_=tot_sb[t][:, :D],
                    func=sqf, accum_out=h1_sb[t][:],
                ).then_inc(ah, 1)
            scalar.wait_ge(st, 16)

        @block.vector
        def _(vector):
            def tadd(out, in0, in1):
                return nc.vector.tensor_tensor(out=out, in0=in0, in1=in1, op=add_op)

            for t in range(NT):
                base = VOPS * t
                s = t % 2
                a3h = []
                g3h = []
                for h in range(2):
                    vector.wait_ge(gs[t][h], 16)
                    g3 = g_sb[t][h].rearrange("p (c e) -> p c e", e=E)
                    a3 = a_sb[s][h].rearrange("p (c e) -> p c e", e=DP)
                    # op 1/2: L1 pair adds (int8 reads in i8 mode)
                    tadd(a3[:, 0:6, :], g3[:, 0:6, 0:DP],
                         g3[:, 6:12, 0:DP]).then_inc(vv, 1)
                    a3h.append(a3)
                    g3h.append(g3)
                for h in range(2):
                    # op 3/4: L2 in-place
                    vector.wait_ge(vv, base + 1 + h)
                    tadd(a3h[h][:, 0:3, :], a3h[h][:, 0:3, :],
                         a3h[h][:, 3:6, :]).then_inc(vv, 1)
                a03, a13 = a3h
                g03, g13 = g3h
                # op 5: 13th rows of both halves folded into a03 block 3
                vector.wait_ge(vv, base + 3)
                tadd(a03[:, 3, :], g03[:, 12, 0:DP],
                     g13[:, 12, 0:DP]).then_inc(vv, 1)
                # op 6: cross-half add into a03 blocks 0..2
                vector.wait_ge(vv, base + 4)
                tadd(a03[:, 0:3, :], a03[:, 0:3, :],
                     a13[:, 0:3, :]).then_inc(vv, 1)
                # ops 7-8: pairwise collapse of the 4 blocks
                vector.wait_ge(vv, base + 6)
                tadd(a03[:, 0, :], a03[:, 0, :], a03[:, 1, :]).then_inc(vv, 1)
                vector.wait_ge(vv, base + 6)
                tadd(a03[:, 2, :], a03[:, 2, :], a03[:, 3, :]).then_inc(vv, 1)
                # op 9: gathered total u = b0 + b2 (1/step units)
                vector.wait_ge(vv, base + 8)
                tadd(u_sb[s][:], a03[:, 0, :], a03[:, 2, :]).then_inc(vv, 1)
                # op 10: tot' = u + dnt'  (both already in 1/step units)
                vector.wait_ge(vv, base + 9)
                vector.wait_ge(ac, t + 1)
                tadd(tot_sb[t][:], u_sb[s][:],
                     dnt16_sb[:, t * DP:(t + 1) * DP]).then_inc(vv, 1)
                # op 11: s'_k = sum_i f'_ik   (Act computes h1' in parallel)
                tv = tot_sb[t][:, :D].rearrange("p (i k) -> p k i", k=K)
                vector.wait_ge(vv, base + 10)
                nc.vector.reduce_sum(
                    out=s8_sb[t][:], in_=tv, axis=mybir.AxisListType.X
                ).then_inc(vv, 1)
                # ops 12-13: h2' = sum_k s'_k^2
                vector.wait_ge(vv, base + 11)
                nc.vector.tensor_tensor(
                    out=sq8_sb[s][:], in0=s8_sb[t][:], in1=s8_sb[t][:],
                    op=mult_op,
                ).then_inc(vv, 1)
                vector.wait_ge(vv, base + 12)
                nc.vector.reduce_sum(
                    out=h2_sb[t][:], in_=sq8_sb[s][:],
                    axis=mybir.AxisListType.X,
                ).then_inc(vv, 1)
                # op 14: r = (h2' - h1') * 0.5*step^2
                vector.wait_ge(vv, base + 13)
                vector.wait_ge(ah, t + 1)
                nc.vector.tensor_scalar(
                    out=rr_sb[t][:], in0=h2_sb[t][:], scalar1=h1_sb[t][:],
                    scalar2=0.5 * step * step, op0=mybir.AluOpType.subtract,
                    op1=mult_op,
                ).then_inc(vv, 1)
                # op 15: out = tot'[312]*step + r
                vector.wait_ge(vv, base + 14)
                nc.vector.tensor_scalar(
                    out=ot4_sb[:, t:t + 1], in0=tot_sb[t][:, D:DW],
                    scalar1=float(step), scalar2=rr_sb[t][:],
                    op0=mult_op, op1=add_op,
                ).then_inc(dn, 1)

    nc.compile()
    _cached[key] = nc
    return nc


def _quant_step(v, w):
    if TABLE_F16:
        return 1.0
    vflat = np.asarray(v, np.float32).reshape(FEAT, D)
    sigma = float(vflat[N_DENSE:].std())
    return sigma * CLIP_SIGMA / 127.0


def _prepare_inputs(inputs, w0, w, v, step):
    dense = np.ascontiguousarray(inputs[:, :N_DENSE].astype(np.float32))
    idx = inputs[:, N_DENSE:].astype(np.int32)
    flat_idx = (N_DENSE + np.arange(N_SPARSE, dtype=np.int32) * ONEHOT)[None, :] + idx

    vflat = np.asarray(v, np.float32).reshape(FEAT, D)
    wflat = np.asarray(w, np.float32).reshape(FEAT)
    if TABLE_F16:
        table = np.zeros((FEAT, E), np.float16)
        table[:, :D] = vflat.astype(np.float16)
        table[:, D] = wflat.astype(np.float16)
    else:
        table = np.zeros((FEAT, E), np.int8)
        table[:, :D] = np.clip(np.rint(vflat / step), -127, 127).astype(np.int8)
        table[:, D] = np.clip(np.rint(wflat / step), -127, 127).astype(np.int8)

    # dense-side v/w/w0 in 1/step units so the gathered int sums add directly
    w0_row = np.zeros((1, DP), np.float32)
    w0_row[0, D] = np.asarray(w0, np.float32).reshape(-1)[0] / step
    vdx_top = np.concatenate(
        [vflat[:N_DENSE] / step, wflat[:N_DENSE, None] / step,
         np.zeros((N_DENSE, 1), np.float32)],
        axis=1,
    ).astype(np.float32)
    vdx = np.ascontiguousarray(np.concatenate([vdx_top, w0_row], axis=0))

    in_maps = []
    for c in range(NCORES):
        sl = slice(c * BC, (c + 1) * BC)
        dnt = np.concatenate(
            [dense[sl].T, np.ones((1, BC), np.float32)], axis=0
        )  # [14, 512]
        # per tile t and half h the gather consumes indices i = f_local*128+p,
        # laid out int16 at [i % 16, i // 16] in the first 16 partitions,
        # replicated 8x down the partitions (one copy per Q7 core)
        fi = flat_idx[sl].astype(np.int16)  # [512, 26]
        blocks = []
        for t in range(NT):
            for h in range(2):
                lin = fi[t * P:(t + 1) * P, 13 * h:13 * (h + 1)].T.reshape(NI // 2)
                blk = lin.reshape(NI // 32, 16).T  # [16, HC]
                blocks.append(np.tile(blk, (8, 1)))  # [128, HC]
        idx_buf = np.ascontiguousarray(np.concatenate(blocks, axis=1))

        mega = np.zeros((P, MB), np.uint8)
        mega[:, MB_IDX:MB_IDX + 1664] = idx_buf.view(np.uint8)
        mega[0:N_DENSE + 1, MB_DNT:MB_DNT + 2048] = (
            np.ascontiguousarray(dnt).view(np.uint8)
        )
        mega[0:N_DENSE + 1, MB_VDX:MB_VDX + 1256] = vdx.view(np.uint8)
        in_maps.append({"table": table, "mega": mega})
    return in_maps


def kernel(**inputs):
    from concourse import bass_utils

    v = np.asarray(inputs["v"])
    w = np.asarray(inputs["w"])
    step = _quant_step(v, w)
    nc = _build_program(step)
    in_maps = _prepare_inputs(
        np.asarray(inputs["inputs"]),
        np.asarray(inputs["w0"]),
        w,
        v,
        step,
    )
    res = bass_utils.run_bass_kernel_spmd(nc, in_maps, core_ids=list(range(NCORES)))
    outs = [
        np.asarray(res.results[c]["out"]).T.reshape(BC, 1) for c in range(NCORES)
    ]
    return np.concatenate(outs, axis=0).astype(np.float32)


# revision 9
# speedup vs baseline: 1.1718x; 1.1718x over previous
"""FFM layer kernel for Trainium2, data-parallel over batch on 8 NeuronCores.

The reference computes, for each sample b:
    x = [dense(13) | onehot(26 fields x 1000)]            # [B, 26013]
    linear = w0 + x @ w                                   # [B, 1]
    field_f = einsum('bf,fik->bik', x, v)                 # [B, 39, 8]
    inter = 0.5*((sum_i field_f)^2.sum(k) - (field_f^2).sum(i,k))
    out = linear + inter

Because x is one-hot in the sparse block, x @ [v|w] is a 26-row gather from
an int8 [26013, 512] table (cols 0..311 = flattened v row / step, col 312 =
w / step, rest pad to a 256 B multiple) plus a tiny fp32 dense [14]x[14,314]
matmul in 1/step units (vdx pre-divided by step on the host; row 13 = ones
row carrying w0/step into col 312).  Each core handles 512 samples as 4
tiles of 128; each tile-half's 13 rows/sample come from ONE dma_gather call
(1664 descriptors) on a rotating SWDGE queue.

Startup is dominated by the gpsimd mlp-library reload (~10.6 us, fixed),
which gates the first gather; the mega input DMA and the dense matmuls
hide under it.  The int8 table halves wire bytes vs fp16 (512 B vs 768 B
rows); quantization uses a 4-sigma clipped step for accuracy margin
(rel_fro ~1.1e-2 vs the 2e-2 gate).

The 26 gathered rows are summed with a pairwise tree on DVE (level 1 reads
int8 at 1 elem/cycle - unavoidable, TRN2 DVE has no 8-bit packing - and the
upper levels run fp16 2x).  Everything stays in integer "1/step" units until
the very end: tot' = u + dnt', h1' = act-square-accum(tot'), r = (h2'-h1')
* 0.5*step^2, out = tot'[312]*step + r.  The [128, 4] per-tile outputs are
stored directly and un-permuted on the host (no PE transpose pass).
"""

import os

import numpy as np

N_DENSE = 13
N_SPARSE = 26
ONEHOT = 1000
FIELD = 39
K = 8
FEAT = N_DENSE + N_SPARSE * ONEHOT  # 26013
B = 4096
NCORES = 8
BC = B // NCORES  # 512 samples per core
P = 128
NT = BC // P  # 4 tiles per core
D = FIELD * K  # 312
DW = D + 1  # 313 (col 312 carries the linear weight)
DP = D + 2  # 314 (even payload width; col 313 zero pad)
NI = N_SPARSE * P  # 3328 gathered rows per tile
HC = (NI // 2) // 16  # idx columns per 13-field half (104)

TABLE_F16 = os.environ.get("K_F16", "0") == "1"
CLIP_SIGMA = float(os.environ.get("K_CLIP", "4.0"))
K_SCR = int(os.environ.get("K_SCR", "65536"))  # SWDGE desc ring carveout
E = 384 if TABLE_F16 else 512  # gathered row width in elements (=bytes for i8)
# SWDGE desc-gen runs at ~8.5 ns/descriptor with ~4 concurrent per-queue
# lanes, and a call's packets only flow once ITS generation finishes - so
# fine-grained calls rotated across queues beat one call per tile-half.
SPLITS = ((0, 5), (5, 4), (9, 4))

# mega input layout (bytes per partition)
MB_IDX = 0                    # [128, 832] int16 = 1664 B
MB_DNT = 1664                 # [14, 512] fp32 = 2048 B
MB_VDX = MB_DNT + 2048        # [14, 314] fp32 = 1256 B
MB = MB_VDX + 1256            # 4968

_cached = {}


def _build_program(step: float):
    key = step
    if key in _cached:
        return _cached[key]

    import concourse.bacc as bacc
    import concourse.mybir as mybir

    nc = bacc.Bacc(
        "TRN2",
        debug=False,
        enable_asserts=False,
        target_bir_lowering=False,
        num_devices=NCORES,
        num_swdge_queues=4,
        dynamic_dma_scratch_size=K_SCR,
    )
    f32 = mybir.dt.float32
    f16 = mybir.dt.float16
    i16 = mybir.dt.int16
    u8 = mybir.dt.uint8
    tdt = f16 if TABLE_F16 else mybir.dt.int8
    add_op = mybir.AluOpType.add
    mult_op = mybir.AluOpType.mult

    table = nc.dram_tensor("table", [FEAT, E], tdt, kind="ExternalInput").ap()
    mega = nc.dram_tensor("mega", [P, MB], u8, kind="ExternalInput").ap()
    out = nc.dram_tensor("out", [P, NT], f32, kind="ExternalOutput").ap()

    mega_sb = nc.alloc_sbuf_tensor("mega_sb", [P, MB], u8).ap()
    idx_sb = mega_sb[:, MB_IDX:MB_IDX + 1664].bitcast(i16)        # [128, 832]
    dnt_sb = mega_sb[0:N_DENSE + 1, MB_DNT:MB_DNT + 2048].bitcast(f32)   # [14, 512]
    vdx_sb = mega_sb[0:N_DENSE + 1, MB_VDX:MB_VDX + 1256].bitcast(f32)   # [14, 314]

    g_sb = [
        [nc.alloc_sbuf_tensor(f"g{t}_{h}", [P, 13 * E], tdt).ap() for h in range(2)]
        for t in range(NT)
    ]
    # two alternating fp16 partial-sum sets (vector-engine-private; explicit
    # vv sems serialize same-engine RAW reuse)
    a_sb = [
        [nc.alloc_sbuf_tensor(f"a{s}_{h}", [P, 6 * DP], f16).ap() for h in range(2)]
        for s in range(2)
    ]
    u_sb = [nc.alloc_sbuf_tensor(f"u{s}", [P, DP], f16).ap() for s in range(2)]
    dnt16_sb = nc.alloc_sbuf_tensor("dnt16", [P, NT * DP], f16).ap()
    tot_sb = [nc.alloc_sbuf_tensor(f"tot{t}", [P, DP], f16).ap() for t in range(NT)]
    s8_sb = [nc.alloc_sbuf_tensor(f"s8_{t}", [P, K], f32).ap() for t in range(NT)]
    sq8_sb = [nc.alloc_sbuf_tensor(f"sq8_{s}", [P, K], f32).ap() for s in range(2)]
    sq_sb = nc.alloc_sbuf_tensor("sq", [P, D], f16).ap()
    h1_sb = [nc.alloc_sbuf_tensor(f"h1_{t}", [P, 1], f32).ap() for t in range(NT)]
    h2_sb = [nc.alloc_sbuf_tensor(f"h2_{t}", [P, 1], f32).ap() for t in range(NT)]
    rr_sb = [nc.alloc_sbuf_tensor(f"rr_{t}", [P, 1], f32).ap() for t in range(NT)]
    ot4_sb = nc.alloc_sbuf_tensor("ot4", [P, NT], f32).ap()
    ps_ps = [nc.alloc_psum_tensor(f"ps{t}", [P, DP], f32).ap() for t in range(NT)]

    io = nc.alloc_semaphore("io")      # mega load x 16
    st = nc.alloc_semaphore("st")      # output store x 16
    # one sem per sub-gather: a DMA sem may only be updated from one SWDGE
    # queue, so the sub-gathers of a tile-half can't share one
    gs = [
        [[nc.alloc_semaphore(f"gs{t}_{h}_{k}") for k in range(len(SPLITS))]
         for h in range(2)]
        for t in range(NT)
    ]
    mm = nc.alloc_semaphore("mm")      # dense matmul done (per tile)
    ac = nc.alloc_semaphore("ac")      # Act PSUM->fp16 copy done (per tile)
    ah = nc.alloc_semaphore("ah")      # Act h1 accum done (per tile)
    dn = nc.alloc_semaphore("dn")      # ot column ready (per tile)
    vv = nc.alloc_semaphore("vv")      # vector-engine same-engine RAW ordering

    VOPS = 13  # vv increments per tile (the final out op increments dn)

    with nc.Block() as block:

        @block.sync
        def _(sync):
            sync.dma_start(mega_sb[:], mega[:]).then_inc(io, 16)
            sync.wait_ge(dn, NT)
            sync.dma_start(out[:], ot4_sb[:]).then_inc(st, 16)
            sync.wait_ge(st, 16)

        @block.gpsimd
        def _(gpsimd):
            from concourse import library_config as lc

            gpsimd.load_library(lc.mlp)
            gpsimd.wait_ge(io, 16)
            qn = 0
            for t in range(NT):
                for h in range(2):
                    g3 = g_sb[t][h].rearrange("p (c e) -> p c e", e=E)
                    col = (2 * t + h) * HC
                    for k, (c0, nf) in enumerate(SPLITS):
                        gpsimd.dma_gather(
                            g3[:, c0:c0 + nf, :],
                            table[:],
                            idx_sb[:, col + c0 * 8:col + (c0 + nf) * 8],
                            nf * P,
                            nf * P,
                            E,
                            single_packet=False,
                            queue_num=qn % 4,
                        ).then_inc(gs[t][h][k], 16)
                        qn += 1

        @block.tensor
        def _(tensor):
            tensor.wait_ge(io, 16)
            for t in range(NT):
                nc.tensor.matmul(
                    out=ps_ps[t][:],
                    lhsT=dnt_sb[:, t * P:(t + 1) * P],
                    rhs=vdx_sb[:],
                    start=True,
                    stop=True,
                ).then_inc(mm, 1)

        @block.scalar
        def _(scalar):
            copyf = mybir.ActivationFunctionType.Copy
            sqf = mybir.ActivationFunctionType.Square
            # downcast the dense-in-1/step-units PSUM to fp16 so the vector
            # add stays in the all-SBUF 2-byte fast path
            for t in range(NT):
                scalar.wait_ge(mm, t + 1)
                nc.scalar.activation(
                    out=dnt16_sb[:, t * DP:(t + 1) * DP], in_=ps_ps[t][:],
                    func=copyf,
                ).then_inc(ac, 1)
            # h1' = sum_i,k f'_ik^2 via the Act accumulator, off the DVE tail
            for t in range(NT):
                scalar.wait_ge(vv, VOPS * t + 9)
                nc.scalar.activation(
                    out=sq_sb[:], in_=tot_sb[t][:, :D],
                    func=sqf, accum_out=h1_sb[t][:],
                ).then_inc(ah, 1)
            scalar.wait_ge(st, 16)

        @block.vector
        def _(vector):
            def tadd(out, in0, in1):
                return nc.vector.tensor_tensor(out=out, in0=in0, in1=in1, op=add_op)

            for t in range(NT):
                base = VOPS * t
                s = t % 2
                a3h = []
                g3h = []
                for h in range(2):
                    for k in range(len(SPLITS)):
                        vector.wait_ge(gs[t][h][k], 16)
                    g3 = g_sb[t][h].rearrange("p (c e) -> p c e", e=E)
                    a3 = a_sb[s][h].rearrange("p (c e) -> p c e", e=DP)
                    # op 1/2: L1 pair adds (int8 reads in i8 mode)
                    tadd(a3[:, 0:6, :], g3[:, 0:6, 0:DP],
                         g3[:, 6:12, 0:DP]).then_inc(vv, 1)
                    a3h.append(a3)
                    g3h.append(g3)
                for h in range(2):
                    # op 3/4: L2 in-place
                    vector.wait_ge(vv, base + 1 + h)
                    tadd(a3h[h][:, 0:3, :], a3h[h][:, 0:3, :],
                         a3h[h][:, 3:6, :]).then_inc(vv, 1)
                a03, a13 = a3h
                g03, g13 = g3h
                # op 5: 13th rows of both halves folded into a03 block 3
                vector.wait_ge(vv, base + 3)
                tadd(a03[:, 3, :], g03[:, 12, 0:DP],
                     g13[:, 12, 0:DP]).then_inc(vv, 1)
                # op 6: cross-half add into a03 blocks 0..2
                vector.wait_ge(vv, base + 4)
                tadd(a03[:, 0:3, :], a03[:, 0:3, :],
                     a13[:, 0:3, :]).then_inc(vv, 1)
                # op 7: pairwise collapse of the 4 blocks, strided pairs in one op
                vector.wait_ge(vv, base + 6)
                tadd(a03[:, 0:4:2, :], a03[:, 0:4:2, :],
                     a03[:, 1:4:2, :]).then_inc(vv, 1)
                # op 8: gathered total u = b0 + b2 (1/step units)
                vector.wait_ge(vv, base + 7)
                tadd(u_sb[s][:], a03[:, 0, :], a03[:, 2, :]).then_inc(vv, 1)
                # op 9: tot' = u + dnt'  (both already in 1/step units)
                vector.wait_ge(vv, base + 8)
                vector.wait_ge(ac, t + 1)
                tadd(tot_sb[t][:], u_sb[s][:],
                     dnt16_sb[:, t * DP:(t + 1) * DP]).then_inc(vv, 1)
                # op 10: s'_k = sum_i f'_ik   (Act computes h1' in parallel)
                tv = tot_sb[t][:, :D].rearrange("p (i k) -> p k i", k=K)
                vector.wait_ge(vv, base + 9)
                nc.vector.reduce_sum(
                    out=s8_sb[t][:], in_=tv, axis=mybir.AxisListType.X
                ).then_inc(vv, 1)
                # ops 11-12: h2' = sum_k s'_k^2
                vector.wait_ge(vv, base + 10)
                nc.vector.tensor_tensor(
                    out=sq8_sb[s][:], in0=s8_sb[t][:], in1=s8_sb[t][:],
                    op=mult_op,
                ).then_inc(vv, 1)
                vector.wait_ge(vv, base + 11)
                nc.vector.reduce_sum(
                    out=h2_sb[t][:], in_=sq8_sb[s][:],
                    axis=mybir.AxisListType.X,
                ).then_inc(vv, 1)
                # op 13: r = (h2' - h1') * 0.5*step^2
                vector.wait_ge(vv, base + 12)
                vector.wait_ge(ah, t + 1)
                nc.vector.tensor_scalar(
                    out=rr_sb[t][:], in0=h2_sb[t][:], scalar1=h1_sb[t][:],
                    scalar2=0.5 * step * step, op0=mybir.AluOpType.subtract,
                    op1=mult_op,
                ).then_inc(vv, 1)
                # final: out = tot'[312]*step + r
                vector.wait_ge(vv, base + 13)
                nc.vector.tensor_scalar(
                    out=ot4_sb[:, t:t + 1], in0=tot_sb[t][:, D:DW],
                    scalar1=float(step), scalar2=rr_sb[t][:],
                    op0=mult_op, op1=add_op,
                ).then_inc(dn, 1)

    nc.compile()
    _cached[key] = nc
    return nc


def _quant_step(v, w):
    if TABLE_F16:
        return 1.0
    vflat = np.asarray(v, np.float32).reshape(FEAT, D)
    sigma = float(vflat[N_DENSE:].std())
    return sigma * CLIP_SIGMA / 127.0


def _prepare_inputs(inputs, w0, w, v, step):
    dense = np.ascontiguousarray(inputs[:, :N_DENSE].astype(np.float32))
    idx = inputs[:, N_DENSE:].astype(np.int32)
    flat_idx = (N_DENSE + np.arange(N_SPARSE, dtype=np.int32) * ONEHOT)[None, :] + idx

    vflat = np.asarray(v, np.float32).reshape(FEAT, D)
    wflat = np.asarray(w, np.float32).reshape(FEAT)
    if TABLE_F16:
        table = np.zeros((FEAT, E), np.float16)
        table[:, :D] = vflat.astype(np.float16)
        table[:, D] = wflat.astype(np.float16)
    else:
        table = np.zeros((FEAT, E), np.int8)
        table[:, :D] = np.clip(np.rint(vflat / step), -127, 127).astype(np.int8)
        table[:, D] = np.clip(np.rint(wflat / step), -127, 127).astype(np.int8)

    # dense-side v/w/w0 in 1/step units so the gathered int sums add directly
    w0_row = np.zeros((1, DP), np.float32)
    w0_row[0, D] = np.asarray(w0, np.float32).reshape(-1)[0] / step
    vdx_top = np.concatenate(
        [vflat[:N_DENSE] / step, wflat[:N_DENSE, None] / step,
         np.zeros((N_DENSE, 1), np.float32)],
        axis=1,
    ).astype(np.float32)
    vdx = np.ascontiguousarray(np.concatenate([vdx_top, w0_row], axis=0))

    in_maps = []
    for c in range(NCORES):
        sl = slice(c * BC, (c + 1) * BC)
        dnt = np.concatenate(
            [dense[sl].T, np.ones((1, BC), np.float32)], axis=0
        )  # [14, 512]
        # per tile t and half h the gather consumes indices i = f_local*128+p,
        # laid out int16 at [i % 16, i // 16] in the first 16 partitions,
        # replicated 8x down the partitions (one copy per Q7 core)
        fi = flat_idx[sl].astype(np.int16)  # [512, 26]
        blocks = []
        for t in range(NT):
            for h in range(2):
                lin = fi[t * P:(t + 1) * P, 13 * h:13 * (h + 1)].T.reshape(NI // 2)
                blk = lin.reshape(NI // 32, 16).T  # [16, HC]
                blocks.append(np.tile(blk, (8, 1)))  # [128, HC]
        idx_buf = np.ascontiguousarray(np.concatenate(blocks, axis=1))

        mega = np.zeros((P, MB), np.uint8)
        mega[:, MB_IDX:MB_IDX + 1664] = idx_buf.view(np.uint8)
        mega[0:N_DENSE + 1, MB_DNT:MB_DNT + 2048] = (
            np.ascontiguousarray(dnt).view(np.uint8)
        )
        mega[0:N_DENSE + 1, MB_VDX:MB_VDX + 1256] = vdx.view(np.uint8)
        in_maps.append({"table": table, "mega": mega})
    return in_maps


def kernel(**inputs):
    from concourse import bass_utils

    v = np.asarray(inputs["v"])
    w = np.asarray(inputs["w"])
    step = _quant_step(v, w)
    nc = _build_program(step)
    in_maps = _prepare_inputs(
        np.asarray(inputs["inputs"]),
        np.asarray(inputs["w0"]),
        w,
        v,
        step,
    )
    res = bass_utils.run_bass_kernel_spmd(nc, in_maps, core_ids=list(range(NCORES)))
    outs = [
        np.asarray(res.results[c]["out"]).T.reshape(BC, 1) for c in range(NCORES)
    ]
    return np.concatenate(outs, axis=0).astype(np.float32)
